# revision 12
# baseline (speedup 1.0000x reference)
"""Trainium2 Bass kernel for the tree-structured dependency encoder.

Reference semantics (per node i, children-first topological order):
    leaf:     z_i = x_i
    internal: mult = max_c params[dep_c] * relu(z_{child_c})   # [D, D]
              z_i  = x_i @ mult                                # [D]
Output: z_root (root = node N-1), shape [1, D].

v2 strategy
-----------
Column sharding across 8 cores (core k owns columns [128k,128k+128) of every
z; zero cross-core traffic).  Per-core layout "colT": dep matrices stored
[128 part = column j, 1024 free = row i'], so a candidate is
tensor_scalar(p, relu(z_child)) with a per-partition scalar.

Per internal node with k edges (HW-validated op menu):
  k>=2: TS mults for edges 1..k-1 (DVE ~0.49us / ACT ~1.16us / Pool),
        serial TT maxes (DVE ~0.70us), last edge fused via DVE
        scalar_tensor_tensor (p*s MAX acc, ~1.25us), gemv fused via STT
        ((acc*1)*xb with accum_out -> z, ~1.25us) or TT + ACT-accum.
  k==1: z = s * (P @ x): single STT ((p*s)*xb, accum) or TT + scaled
        ACT-accum (scale=relu'd child as the per-partition activation scale).
  relu: tiny DVE tensor_scalar_max on [128,1].
Root z [128,1] is transposed to a [1,128] row via PE matmul against an
identity so the final DRAM store is one contiguous 512B descriptor (a
[128,1] column store pays ~6us of scattered-completion tail).

Engine choice + emission order come from a HEFT-style list scheduler over
the op DAG with measured per-op costs; Tile then inserts all semaphores.
"""

import os
import numpy as np

N_CORES = 8
D = 1024
DC = D // N_CORES  # 128 columns per core

PARAM_CHUNK = 4        # labels per param DMA
XB_CHUNK = 4           # x rows per xb-broadcast DMA
CROSS_LAT = 100.0      # ns, cross-engine sem propagation in the sim
DMA_LAT = 1500.0       # ns, DMA completion latency beyond transfer time
POOL_MUL_OK = os.environ.get("POOL_MUL_OK", "1") == "1"

# per-op costs (ns) on each engine, [128,1024] tiles unless noted
COST = {
    "ts":   {"DVE": 490.0, "ACT": 1160.0, "POOL": 1300.0},
    "tt":   {"DVE": 700.0},              # max merge: DVE only (no Pool max)
    "gm":   {"DVE": 700.0, "POOL": 1750.0},  # gemv elementwise acc*xb
    "gr":   {"ACT": 1450.0, "DVE": 1250.0},  # free-axis sum of gm
    "sttm": {"DVE": 1270.0},             # fused (p*s) MAX acc
    "sttg": {"DVE": 1270.0},             # fused (acc*1)*xb + accum
    "sttg1": {"DVE": 1270.0},            # fused ((p*s)*xb, accum) k=1 node
    "relu": {"DVE": 130.0, "ACT": 400.0},
    "pe":   {"PE": 300.0},               # root transpose matmul
    "cp":   {"DVE": 320.0},              # PSUM->SBUF [1,128] copy
}

_CACHE = {}


def _schedule(children_idx, children_dep, children_mask):
    """Prune to the root's ancestor cone and build the edge schedule."""
    n = children_idx.shape[0]
    root = n - 1
    ci = np.asarray(children_idx, dtype=np.int64)
    cd = np.asarray(children_dep, dtype=np.int64)
    cm = np.asarray(children_mask, dtype=bool)

    needed = set()
    stack = [root]
    while stack:
        i = stack.pop()
        if i in needed:
            continue
        needed.add(i)
        for c in range(ci.shape[1]):
            if cm[i, c]:
                stack.append(int(ci[i, c]))

    order = sorted(needed)
    internal, leaves = [], []
    edges = {}
    for i in order:
        if not cm[i].any():
            leaves.append(i)
            continue
        internal.append(i)
        seen = set()
        elist = []
        for c in range(ci.shape[1]):
            if cm[i, c]:
                key = (int(ci[i, c]), int(cd[i, c]))
                if key not in seen:
                    seen.add(key)
                    elist.append(key)
        edges[i] = elist

    depth = {}
    for i in order:
        if i not in edges:
            depth[i] = 0
        else:
            depth[i] = 1 + max(depth[c] for c, _ in edges[i])

    # the single longest root-to-leaf chain ("critical path" nodes)
    path = set()
    cur = root
    while cur in edges:
        path.add(cur)
        cur = max((c for c, _ in edges[cur]), key=lambda c: depth[c])

    labels = []
    lab2slot = {}
    for i in internal:
        for _, d2 in edges[i]:
            if d2 not in lab2slot:
                lab2slot[d2] = len(labels)
                labels.append(d2)

    return {
        "root": root,
        "order": order,
        "internal": internal,
        "leaves": leaves,
        "edges": edges,
        "labels": labels,
        "lab2slot": lab2slot,
        "depth": depth,
        "path": path,
    }


def _legalize_single_wait(nc):
    """Split multi-wait instructions: this walrus allows 1 sync wait/inst."""
    from concourse import mybir

    for bb in nc.main_func.blocks:
        new_list = []
        for inst in bb.instructions:
            si = inst.sync_info
            if si is not None and si.on_wait and len(si.on_wait) > 1:
                waits = list(si.on_wait)
                for w in waits[:-1]:
                    nop = mybir.InstNoOp(
                        name=nc.get_next_instruction_name(), ins=[], outs=[]
                    )
                    nop.engine = inst.engine
                    nop.sync_info = mybir.SyncInfo(on_wait=[w], on_update=[])
                    new_list.append(nop)
                inst.sync_info = mybir.SyncInfo(
                    on_wait=[waits[-1]], on_update=list(si.on_update)
                )
            new_list.append(inst)
        bb.instructions = new_list


class _Op:
    __slots__ = ("oid", "kind", "engines", "deps", "info", "rank",
                 "engine", "start", "dur")

    def __init__(self, oid, kind, engines, deps, info):
        self.oid = oid
        self.kind = kind
        self.engines = engines
        self.deps = deps
        self.info = info
        self.rank = 0.0
        self.engine = None
        self.start = 0.0
        self.dur = 0.0


def _build_op_graph(sched):
    """Op DAG for the whole kernel. Returns (ops, meta) where meta carries
    the chunk layouts needed by both the emitter and the host prep."""
    internal = sched["internal"]
    leaves = sched["leaves"]
    edges = sched["edges"]
    lab2slot = sched["lab2slot"]
    labels = sched["labels"]
    depth = sched["depth"]
    root = sched["root"]

    ops = []

    def add(kind, engines, deps, **info):
        op = _Op(len(ops), kind, engines, deps, info)
        ops.append(op)
        return op.oid

    # --- param chunk DMAs, in first-use order
    n_chunks = (len(labels) + PARAM_CHUNK - 1) // PARAM_CHUNK
    chunk_of = {}   # slot -> (chunk_id, offset_in_chunk)
    chunk_sizes = []
    for ci in range(n_chunks):
        sl = labels[ci * PARAM_CHUNK:(ci + 1) * PARAM_CHUNK]
        chunk_sizes.append(len(sl))
        for j in range(len(sl)):
            chunk_of[ci * PARAM_CHUNK + j] = (ci, j)
    pdma = {}
    for ci in range(n_chunks):
        nbytes = DC * D * 2 * chunk_sizes[ci]
        pdma[ci] = add("dma", ("DMA",), (), what="param", chunk=ci,
                       bytes=nbytes)

    # --- xb broadcast DMAs (XB_CHUNK internal-node x rows per DMA)
    iloc = {node: t for t, node in enumerate(internal)}
    n_xch = (len(internal) + XB_CHUNK - 1) // XB_CHUNK
    xdma = {}
    for ci in range(n_xch):
        rows = list(range(ci * XB_CHUNK, min((ci + 1) * XB_CHUNK,
                                             len(internal))))
        nbytes = DC * D * 2 * len(rows)
        oid = add("dma", ("DMA",), (), what="xb", chunk=ci, rows=rows,
                  bytes=nbytes)
        for t in rows:
            xdma[t] = oid

    # --- leaf relus arrive with one small DMA
    rl_dma = add("dma", ("DMA",), (), what="rl", bytes=DC * 4 * max(
        len(leaves), 1))
    id_dma = add("dma", ("DMA",), (), what="ident", bytes=DC * DC * 4)

    rt_op = {leaf: rl_dma for leaf in leaves}  # producer op of relu'd value
    z_op = {}
    path = sched["path"]
    ts_eng = ("DVE", "ACT", "POOL") if POOL_MUL_OK else ("DVE", "ACT")

    for i in internal:
        elist = sorted(edges[i], key=lambda e: depth[e[0]])
        k = len(elist)
        t_i = iloc[i]
        on_path = i in path
        if k == 1:
            c, dlab = elist[0]
            s = lab2slot[dlab]
            pc, _ = chunk_of[s]
            if on_path:
                # single fused STT: ((p*s)*xb, accum) -> z
                z_op[i] = add("sttg1", ("DVE",),
                              (pdma[pc], xdma[t_i], rt_op[c]),
                              node=i, slot=s, child=c)
            else:
                # gm = p*xb off-path, then scaled ACT reduce (scale=relu'd z)
                gm = add("gm", ("DVE", "POOL") if POOL_MUL_OK else ("DVE",),
                         (pdma[pc], xdma[t_i]), node=i, slot=s, k1=True)
                z_op[i] = add("gr", ("ACT",), (gm, rt_op[c]),
                              node=i, k1=True, child=c)
        elif on_path:
            # TS mults for edges 0..k-2; last edge fused into the max via STT
            tops = []
            for c, dlab in elist[:-1]:
                s = lab2slot[dlab]
                pc, _ = chunk_of[s]
                tops.append(add("ts", ts_eng, (pdma[pc], rt_op[c]),
                                node=i, slot=s, child=c))
            acc = tops[0]
            for t2 in tops[1:]:
                acc = add("tt", ("DVE",), (acc, t2), node=i)
            c, dlab = elist[-1]
            s = lab2slot[dlab]
            pc, _ = chunk_of[s]
            accf = add("sttm", ("DVE",), (acc, pdma[pc], rt_op[c]),
                       node=i, slot=s, child=c)
            z_op[i] = add("sttg", ("DVE",), (accf, xdma[t_i]), node=i)
        else:
            tops = []
            for c, dlab in elist:
                s = lab2slot[dlab]
                pc, _ = chunk_of[s]
                tops.append(add("ts", ts_eng, (pdma[pc], rt_op[c]),
                                node=i, slot=s, child=c))
            acc = tops[0]
            for t2 in tops[1:]:
                acc = add("tt", ("DVE",), (acc, t2), node=i)
            gm = add("gm", ("DVE", "POOL") if POOL_MUL_OK else ("DVE",),
                     (acc, xdma[t_i]), node=i, k1=False)
            z_op[i] = add("gr", ("ACT", "DVE"), (gm,), node=i, k1=False)
        if i != root:
            rt_op[i] = add("relu", ("DVE", "ACT"), (z_op[i],), node=i)

    pe = add("pe", ("PE",), (z_op[root], id_dma))
    cp = add("cp", ("DVE",), (pe,))
    out = add("dma", ("DMA",), (cp,), what="out", bytes=DC * 4)

    meta = {
        "chunk_of": chunk_of,
        "chunk_sizes": chunk_sizes,
        "n_chunks": n_chunks,
        "n_xch": n_xch,
        "iloc": iloc,
        "z_op": z_op,
    }
    return ops, meta


def _heft(ops):
    """List-schedule the op DAG; sets engine/start on each op, returns
    makespan. Engines: DVE/ACT/POOL/PE compute queues, SP DMA-issue queue,
    DMA transfer pipe."""
    succs = [[] for _ in ops]
    for op in ops:
        for d in op.deps:
            succs[d].append(op.oid)

    def mincost(op):
        if op.kind == "dma":
            return op.info["bytes"] / 360.0 + 600.0
        return min(COST[op.kind].values())

    # upward rank (longest path to sink)
    for op in reversed(ops):
        op.rank = mincost(op) + max(
            (ops[s].rank for s in succs[op.oid]), default=0.0)

    indeg = [len(op.deps) for op in ops]
    import heapq
    ready = [(-op.rank, op.oid) for op in ops if not op.deps]
    heapq.heapify(ready)
    free = {"DVE": 0.0, "ACT": 0.0, "POOL": 0.0, "PE": 0.0,
            "SP": 0.0, "DMA": 0.0}
    load = {k: 0.0 for k in free}
    done_t = [0.0] * len(ops)
    makespan = 0.0
    while ready:
        _, oid = heapq.heappop(ready)
        op = ops[oid]
        if op.kind == "dma":
            ready_t = max((done_t[d] for d in op.deps), default=0.0)
            issue = max(free["SP"], ready_t) + 600.0
            start = max(free["DMA"], issue)
            dur = op.info["bytes"] / 360.0
            free["SP"] = issue
            free["DMA"] = start + dur
            op.engine, op.start, op.dur = "DMA", start, dur
            done_t[oid] = start + dur + DMA_LAT
        else:
            best = None
            for eng in op.engines:
                ready_t = max(
                    (done_t[d] + (0.0 if ops[d].engine == eng else CROSS_LAT)
                     for d in op.deps), default=0.0)
                start = max(free[eng], ready_t)
                dur = COST[op.kind][eng]
                fin = start + dur
                # pressure term steers work off saturated engines even when
                # they are momentarily idle (greedy min-finish alone floods
                # the fastest engine)
                score = fin + 0.55 * (load[eng] + dur)
                if best is None or score < best[0]:
                    best = (score, fin, eng, start)
            _, fin, eng, start = best
            op.engine, op.start, op.dur = eng, start, fin - start
            free[eng] = fin
            load[eng] += op.dur
            done_t[oid] = fin
        makespan = max(makespan, done_t[oid])
        for s in succs[oid]:
            indeg[s] -= 1
            if indeg[s] == 0:
                heapq.heappush(ready, (-ops[s].rank, s))
    return makespan


def _build_program(sched, legalize=True):
    import concourse.bass as bass
    import concourse.tile as tile
    from concourse import mybir

    f32 = mybir.dt.float32
    bf16 = mybir.dt.bfloat16
    MUL = mybir.AluOpType.mult
    MAX = mybir.AluOpType.max
    COPY = mybir.ActivationFunctionType.Copy

    ops, meta = _build_op_graph(sched)
    makespan = _heft(ops)
    order = sorted(range(len(ops)), key=lambda o: (ops[o].start, o))

    internal = sched["internal"]
    leaves = sched["leaves"]
    lab2slot = sched["lab2slot"]
    root = sched["root"]
    iloc = meta["iloc"]
    chunk_of = meta["chunk_of"]
    chunk_sizes = meta["chunk_sizes"]
    n_leaves = max(len(leaves), 1)

    nc = bass.Bass()
    pts = {ci: nc.dram_tensor(f"pc{ci}", [DC, chunk_sizes[ci] * D], bf16,
                              kind="ExternalInput")
           for ci in range(meta["n_chunks"])}
    xr = nc.dram_tensor("xr", [len(internal), D], bf16, kind="ExternalInput")
    rl = nc.dram_tensor("rl", [DC, n_leaves], f32, kind="ExternalInput")
    idm = nc.dram_tensor("idm", [DC, DC], f32, kind="ExternalInput")
    zr = nc.dram_tensor("zr", [1, DC], f32, kind="ExternalOutput")

    with tile.TileContext(nc) as tc:
        with (
            tc.tile_pool(name="pparams", bufs=1) as ppool,
            tc.tile_pool(name="pwork", bufs=1) as wpool,
            tc.tile_pool(name="psmall", bufs=1) as spool,
            tc.tile_pool(name="ppsum", bufs=1, space="PSUM") as psum_pool,
        ):
            pc_t = {}
            xb_t = {}      # per internal-node index t -> AP slice
            rl_t = None
            id_t = None
            rt = {}        # node -> [DC,1] f32 AP of relu'd z
            zt = {}        # node -> [DC,1] f32 tile
            tt_of = {}     # mult op id -> tile
            acc_of = {}    # merge/sttm op id -> tile
            zrow_p = None

            def slot_ap(s):
                ci, off = chunk_of[s]
                return pc_t[ci][:, off * D:(off + 1) * D]

            for oid in order:
                op = ops[oid]
                k = op.kind
                if k == "dma":
                    what = op.info["what"]
                    if what == "param":
                        ci = op.info["chunk"]
                        t = ppool.tile([DC, chunk_sizes[ci] * D], bf16,
                                       tag=f"pc{ci}", name=f"pc{ci}")
                        nc.sync.dma_start(out=t, in_=pts[ci][:, :])
                        pc_t[ci] = t
                    elif what == "xb":
                        rows = op.info["rows"]
                        w = len(rows)
                        xt = ppool.tile([DC, w * D], bf16,
                                        tag=f"xc{op.info['chunk']}",
                                        name=f"xc{op.info['chunk']}")
                        src = xr[rows[0]:rows[0] + w, :]
                        bsrc = bass.AP(
                            tensor=src.tensor, offset=src.offset,
                            ap=[[0, DC]] + list(src.ap),
                        )
                        nc.sync.dma_start(out=xt, in_=bsrc)
                        for j, t in enumerate(rows):
                            xb_t[t] = xt[:, j * D:(j + 1) * D]
                    elif what == "rl":
                        rl_t = spool.tile([DC, n_leaves], f32, tag="rl",
                                          name="rl_t")
                        nc.sync.dma_start(out=rl_t, in_=rl[:, :])
                        for li, leaf in enumerate(leaves):
                            rt[leaf] = rl_t[:, li:li + 1]
                    elif what == "ident":
                        id_t = spool.tile([DC, DC], f32, tag="idm",
                                          name="id_t")
                        nc.sync.dma_start(out=id_t, in_=idm[:, :])
                    elif what == "out":
                        nc.sync.dma_start(out=zr[:, :], in_=zrow_s)
                elif k == "ts":
                    t = wpool.tile([DC, D], bf16, tag="t", name="t", bufs=10)
                    p_ap = slot_ap(op.info["slot"])
                    s_ap = rt[op.info["child"]]
                    if op.engine == "ACT":
                        nc.scalar.mul(t, p_ap, s_ap)
                    elif op.engine == "POOL":
                        nc.gpsimd.tensor_scalar_mul(t, p_ap, s_ap)
                    else:
                        nc.vector.tensor_scalar_mul(t, p_ap, s_ap)
                    tt_of[oid] = t
                elif k == "tt":
                    a = tt_of.get(op.deps[0], acc_of.get(op.deps[0]))
                    b = tt_of.get(op.deps[1], acc_of.get(op.deps[1]))
                    m = wpool.tile([DC, D], bf16, tag="m", name="m", bufs=6)
                    nc.vector.tensor_tensor(out=m, in0=a, in1=b, op=MAX)
                    acc_of[oid] = m
                elif k == "sttm":
                    acc = tt_of.get(op.deps[0], acc_of.get(op.deps[0]))
                    p_ap = slot_ap(op.info["slot"])
                    s_ap = rt[op.info["child"]]
                    af = wpool.tile([DC, D], bf16, tag="af", name="af",
                                    bufs=6)
                    nc.vector.scalar_tensor_tensor(
                        out=af, in0=p_ap, scalar=s_ap, in1=acc,
                        op0=MUL, op1=MAX)
                    acc_of[oid] = af
                elif k in ("sttg", "sttg1"):
                    i = op.info["node"]
                    z = spool.tile([DC, 1], f32, tag=f"z{i}", name=f"z{i}")
                    g = wpool.tile([DC, D], bf16, tag="g", name="g", bufs=4)
                    if k == "sttg":
                        acc = acc_of[op.deps[0]]
                        nc.vector.scalar_tensor_tensor(
                            out=g, in0=acc, scalar=1.0, in1=xb_t[iloc[i]],
                            op0=MUL, op1=MUL, accum_out=z)
                    else:
                        p_ap = slot_ap(op.info["slot"])
                        s_ap = rt[op.info["child"]]
                        nc.vector.scalar_tensor_tensor(
                            out=g, in0=p_ap, scalar=s_ap, in1=xb_t[iloc[i]],
                            op0=MUL, op1=MUL, accum_out=z)
                    zt[i] = z
                elif k == "gm":
                    i = op.info["node"]
                    if op.info["k1"]:
                        a = slot_ap(op.info["slot"])
                    else:
                        a = acc_of.get(op.deps[0], tt_of.get(op.deps[0]))
                    g = wpool.tile([DC, D], bf16, tag="g", name="g", bufs=4)
                    if op.engine == "POOL":
                        nc.gpsimd.tensor_tensor(
                            out=g, in0=a, in1=xb_t[iloc[i]], op=MUL)
                    else:
                        nc.vector.tensor_tensor(
                            out=g, in0=a, in1=xb_t[iloc[i]], op=MUL)
                    tt_of[oid] = g
                elif k == "gr":
                    i = op.info["node"]
                    g = tt_of[op.deps[0]]
                    z = spool.tile([DC, 1], f32, tag=f"z{i}", name=f"z{i}")
                    if op.engine == "DVE":
                        nc.vector.tensor_reduce(
                            z, g, axis=mybir.AxisListType.X,
                            op=mybir.AluOpType.add)
                    else:
                        scr = wpool.tile([DC, D], bf16, tag="scr",
                                         name="scr", bufs=2)
                        if op.info["k1"]:
                            nc.scalar.activation(
                                scr, g, COPY, scale=rt[op.info["child"]],
                                accum_out=z)
                        else:
                            nc.scalar.activation(scr, g, COPY, accum_out=z)
                    zt[i] = z
                elif k == "relu":
                    i = op.info["node"]
                    r = spool.tile([DC, 1], f32, tag=f"r{i}", name=f"r{i}")
                    nc.vector.tensor_scalar_max(r, zt[i], 0.0)
                    rt[i] = r
                elif k == "pe":
                    zrow_p = psum_pool.tile([1, DC], f32, tag="zrow",
                                            name="zrow")
                    nc.tensor.matmul(zrow_p, zt[root], id_t)
                elif k == "cp":
                    zrow_s = spool.tile([1, DC], f32, tag="zrow_s",
                                        name="zrow_s")
                    nc.vector.tensor_copy(zrow_s, zrow_p)

    if legalize:
        _legalize_single_wait(nc)
    return nc, meta, makespan


def _prepare(embeddings, params, children_idx, children_dep, children_mask,
             legalize=True):
    import ml_dtypes

    emb = np.ascontiguousarray(np.asarray(embeddings, dtype=np.float32))
    par = np.asarray(params, dtype=np.float32)
    sched = _schedule(children_idx, children_dep, children_mask)

    key = (
        legalize,
        tuple(sched["order"]),
        tuple(sched["labels"]),
        tuple((i, tuple(e)) for i, e in sched["edges"].items()),
    )
    if key in _CACHE:
        nc, meta = _CACHE[key]
    else:
        nc, meta, _ = _build_program(sched, legalize=legalize)
        _CACHE[key] = (nc, meta)

    internal = sched["internal"]
    leaves = sched["leaves"]
    labels = sched["labels"]
    n_leaves = max(len(leaves), 1)
    chunk_sizes = meta["chunk_sizes"]

    xr = np.ascontiguousarray(emb[internal]).astype(ml_dtypes.bfloat16)
    ident = np.eye(DC, dtype=np.float32)
    in_maps = []
    for kcore in range(N_CORES):
        cols = slice(kcore * DC, (kcore + 1) * DC)
        m = {"xr": xr, "idm": ident}
        off = 0
        for ci, csz in enumerate(chunk_sizes):
            sl = labels[off:off + csz]
            off += csz
            # [csz, D, DC] -> [DC, csz*D] colT layout
            blk = par[sl][:, :, cols].transpose(2, 0, 1).reshape(DC, csz * D)
            m[f"pc{ci}"] = np.ascontiguousarray(blk).astype(ml_dtypes.bfloat16)
        rl_k = np.zeros((DC, n_leaves), dtype=np.float32)
        if leaves:
            rl_k[:, :len(leaves)] = np.maximum(emb[leaves][:, cols], 0.0).T
        m["rl"] = rl_k
        in_maps.append(m)
    return sched, nc, in_maps


def _run(embeddings, params, children_idx, children_dep, children_mask,
         trace=False):
    emb = np.asarray(embeddings, dtype=np.float32)
    cm = np.asarray(children_mask, dtype=bool)
    root = emb.shape[0] - 1
    if not cm[root].any():  # degenerate: root is a leaf
        return emb[root:root + 1].copy(), None

    from concourse.bass_utils import run_bass_kernel_spmd

    sched, nc, in_maps = _prepare(
        embeddings, params, children_idx, children_dep, children_mask
    )
    bkr = run_bass_kernel_spmd(
        nc, in_maps, core_ids=list(range(N_CORES)), trace=trace
    )
    out = np.concatenate(
        [bkr.results[k]["zr"].reshape(DC) for k in range(N_CORES)]
    ).reshape(1, D)
    return out.astype(np.float32), bkr


def kernel(embeddings, params, children_idx, children_dep, children_mask):
    out, _ = _run(embeddings, params, children_idx, children_dep,
                  children_mask)
    return out


def run_traced(embeddings, params, children_idx, children_dep, children_mask):
    return _run(embeddings, params, children_idx, children_dep,
                children_mask, trace=True)


# revision 13
# speedup vs baseline: 4.4300x; 4.4300x over previous
"""Trainium2 Bass kernel for the tree-structured dependency encoder.

Reference semantics (per node i, children-first topological order):
    leaf:     z_i = x_i
    internal: mult = max_c params[dep_c] * relu(z_{child_c})   # [D, D]
              z_i  = x_i @ mult                                # [D]
Output: z_root (root = node N-1), shape [1, D].

v2 strategy
-----------
Column sharding across 8 cores (core k owns columns [128k,128k+128) of every
z; zero cross-core traffic).  Per-core layout "colT": dep matrices stored
[128 part = column j, 1024 free = row i'], so a candidate is
tensor_scalar(p, relu(z_child)) with a per-partition scalar.

Per internal node with k edges (HW-validated op menu):
  k>=2: TS mults for edges 1..k-1 (DVE ~0.49us / ACT ~1.16us / Pool),
        serial TT maxes (DVE ~0.70us), last edge fused via DVE
        scalar_tensor_tensor (p*s MAX acc, ~1.25us), gemv fused via STT
        ((acc*1)*xb with accum_out -> z, ~1.25us) or TT + ACT-accum.
  k==1: z = s * (P @ x): single STT ((p*s)*xb, accum) or TT + scaled
        ACT-accum (scale=relu'd child as the per-partition activation scale).
  relu: tiny DVE tensor_scalar_max on [128,1].
Root z [128,1] is transposed to a [1,128] row via PE matmul against an
identity so the final DRAM store is one contiguous 512B descriptor (a
[128,1] column store pays ~6us of scattered-completion tail).

Engine choice + emission order come from a HEFT-style list scheduler over
the op DAG with measured per-op costs; Tile then inserts all semaphores.
"""

import os
import numpy as np

N_CORES = 8
D = 1024
DC = D // N_CORES  # 128 columns per core

PARAM_CHUNK = 4        # labels per param DMA
XB_CHUNK = 4           # x rows per xb-broadcast DMA
CROSS_LAT = 100.0      # ns, cross-engine sem propagation in the sim
DMA_LAT = 1500.0       # ns, DMA completion latency beyond transfer time
# GpSimd measured ~15us per [128,1024] tensor_scalar AND its SBUF port
# traffic slows concurrent DVE ops 3-7x -- keep Pool off the big tiles.
POOL_MUL_OK = os.environ.get("POOL_MUL_OK", "0") == "1"

# per-op costs (ns) on each engine, [128,1024] tiles unless noted
COST = {
    "ts":   {"DVE": 490.0, "ACT": 1160.0, "POOL": 1300.0},
    "tt":   {"DVE": 700.0},              # max merge: DVE only (no Pool max)
    "gm":   {"DVE": 700.0, "POOL": 1750.0},  # gemv elementwise acc*xb
    "gr":   {"ACT": 1450.0, "DVE": 1250.0},  # free-axis sum of gm
    "sttm": {"DVE": 1270.0},             # fused (p*s) MAX acc
    "sttg": {"DVE": 1270.0},             # fused (acc*1)*xb + accum
    "sttg1": {"DVE": 1270.0},            # fused ((p*s)*xb, accum) k=1 node
    "relu": {"DVE": 130.0, "ACT": 400.0},
    "pe":   {"PE": 300.0},               # root transpose matmul
    "cp":   {"DVE": 320.0},              # PSUM->SBUF [1,128] copy
}

_CACHE = {}


def _schedule(children_idx, children_dep, children_mask):
    """Prune to the root's ancestor cone and build the edge schedule."""
    n = children_idx.shape[0]
    root = n - 1
    ci = np.asarray(children_idx, dtype=np.int64)
    cd = np.asarray(children_dep, dtype=np.int64)
    cm = np.asarray(children_mask, dtype=bool)

    needed = set()
    stack = [root]
    while stack:
        i = stack.pop()
        if i in needed:
            continue
        needed.add(i)
        for c in range(ci.shape[1]):
            if cm[i, c]:
                stack.append(int(ci[i, c]))

    order = sorted(needed)
    internal, leaves = [], []
    edges = {}
    for i in order:
        if not cm[i].any():
            leaves.append(i)
            continue
        internal.append(i)
        seen = set()
        elist = []
        for c in range(ci.shape[1]):
            if cm[i, c]:
                key = (int(ci[i, c]), int(cd[i, c]))
                if key not in seen:
                    seen.add(key)
                    elist.append(key)
        edges[i] = elist

    depth = {}
    for i in order:
        if i not in edges:
            depth[i] = 0
        else:
            depth[i] = 1 + max(depth[c] for c, _ in edges[i])

    # the single longest root-to-leaf chain ("critical path" nodes)
    path = set()
    cur = root
    while cur in edges:
        path.add(cur)
        cur = max((c for c, _ in edges[cur]), key=lambda c: depth[c])

    labels = []
    lab2slot = {}
    for i in internal:
        for _, d2 in edges[i]:
            if d2 not in lab2slot:
                lab2slot[d2] = len(labels)
                labels.append(d2)

    return {
        "root": root,
        "order": order,
        "internal": internal,
        "leaves": leaves,
        "edges": edges,
        "labels": labels,
        "lab2slot": lab2slot,
        "depth": depth,
        "path": path,
    }


def _legalize_single_wait(nc):
    """Split multi-wait instructions: this walrus allows 1 sync wait/inst."""
    from concourse import mybir

    for bb in nc.main_func.blocks:
        new_list = []
        for inst in bb.instructions:
            si = inst.sync_info
            if si is not None and si.on_wait and len(si.on_wait) > 1:
                waits = list(si.on_wait)
                for w in waits[:-1]:
                    nop = mybir.InstNoOp(
                        name=nc.get_next_instruction_name(), ins=[], outs=[]
                    )
                    nop.engine = inst.engine
                    nop.sync_info = mybir.SyncInfo(on_wait=[w], on_update=[])
                    new_list.append(nop)
                inst.sync_info = mybir.SyncInfo(
                    on_wait=[waits[-1]], on_update=list(si.on_update)
                )
            new_list.append(inst)
        bb.instructions = new_list


class _Op:
    __slots__ = ("oid", "kind", "engines", "deps", "info", "rank",
                 "engine", "start", "dur")

    def __init__(self, oid, kind, engines, deps, info):
        self.oid = oid
        self.kind = kind
        self.engines = engines
        self.deps = deps
        self.info = info
        self.rank = 0.0
        self.engine = None
        self.start = 0.0
        self.dur = 0.0


def _build_op_graph(sched):
    """Op DAG for the whole kernel. Returns (ops, meta) where meta carries
    the chunk layouts needed by both the emitter and the host prep."""
    internal = sched["internal"]
    leaves = sched["leaves"]
    edges = sched["edges"]
    lab2slot = sched["lab2slot"]
    labels = sched["labels"]
    depth = sched["depth"]
    root = sched["root"]

    ops = []

    def add(kind, engines, deps, **info):
        op = _Op(len(ops), kind, engines, deps, info)
        ops.append(op)
        return op.oid

    # --- param chunk DMAs, in first-use order
    n_chunks = (len(labels) + PARAM_CHUNK - 1) // PARAM_CHUNK
    chunk_of = {}   # slot -> (chunk_id, offset_in_chunk)
    chunk_sizes = []
    for ci in range(n_chunks):
        sl = labels[ci * PARAM_CHUNK:(ci + 1) * PARAM_CHUNK]
        chunk_sizes.append(len(sl))
        for j in range(len(sl)):
            chunk_of[ci * PARAM_CHUNK + j] = (ci, j)
    pdma = {}
    for ci in range(n_chunks):
        nbytes = DC * D * 2 * chunk_sizes[ci]
        pdma[ci] = add("dma", ("DMA",), (), what="param", chunk=ci,
                       bytes=nbytes)

    # --- xb broadcast DMAs (XB_CHUNK internal-node x rows per DMA)
    iloc = {node: t for t, node in enumerate(internal)}
    n_xch = (len(internal) + XB_CHUNK - 1) // XB_CHUNK
    xdma = {}
    for ci in range(n_xch):
        rows = list(range(ci * XB_CHUNK, min((ci + 1) * XB_CHUNK,
                                             len(internal))))
        nbytes = DC * D * 2 * len(rows)
        oid = add("dma", ("DMA",), (), what="xb", chunk=ci, rows=rows,
                  bytes=nbytes)
        for t in rows:
            xdma[t] = oid

    # --- leaf relus arrive with one small DMA
    rl_dma = add("dma", ("DMA",), (), what="rl", bytes=DC * 4 * max(
        len(leaves), 1))
    id_dma = add("dma", ("DMA",), (), what="ident", bytes=DC * DC * 4)

    rt_op = {leaf: rl_dma for leaf in leaves}  # producer op of relu'd value
    z_op = {}
    path = sched["path"]
    ts_eng = ("DVE", "ACT", "POOL") if POOL_MUL_OK else ("DVE", "ACT")

    for i in internal:
        elist = sorted(edges[i], key=lambda e: depth[e[0]])
        k = len(elist)
        t_i = iloc[i]
        on_path = i in path
        if k == 1:
            c, dlab = elist[0]
            s = lab2slot[dlab]
            pc, _ = chunk_of[s]
            if on_path:
                # single fused STT: ((p*s)*xb, accum) -> z
                z_op[i] = add("sttg1", ("DVE",),
                              (pdma[pc], xdma[t_i], rt_op[c]),
                              node=i, slot=s, child=c)
            else:
                # gm = p*xb off-path, then scaled ACT reduce (scale=relu'd z)
                gm = add("gm", ("DVE", "POOL") if POOL_MUL_OK else ("DVE",),
                         (pdma[pc], xdma[t_i]), node=i, slot=s, k1=True)
                z_op[i] = add("gr", ("ACT",), (gm, rt_op[c]),
                              node=i, k1=True, child=c)
        elif on_path:
            # TS mults for edges 0..k-2; last edge fused into the max via STT
            tops = []
            for c, dlab in elist[:-1]:
                s = lab2slot[dlab]
                pc, _ = chunk_of[s]
                tops.append(add("ts", ts_eng, (pdma[pc], rt_op[c]),
                                node=i, slot=s, child=c))
            acc = tops[0]
            for t2 in tops[1:]:
                acc = add("tt", ("DVE",), (acc, t2), node=i)
            c, dlab = elist[-1]
            s = lab2slot[dlab]
            pc, _ = chunk_of[s]
            accf = add("sttm", ("DVE",), (acc, pdma[pc], rt_op[c]),
                       node=i, slot=s, child=c)
            z_op[i] = add("sttg", ("DVE",), (accf, xdma[t_i]), node=i)
        else:
            tops = []
            for c, dlab in elist:
                s = lab2slot[dlab]
                pc, _ = chunk_of[s]
                tops.append(add("ts", ts_eng, (pdma[pc], rt_op[c]),
                                node=i, slot=s, child=c))
            acc = tops[0]
            for t2 in tops[1:]:
                acc = add("tt", ("DVE",), (acc, t2), node=i)
            gm = add("gm", ("DVE", "POOL") if POOL_MUL_OK else ("DVE",),
                     (acc, xdma[t_i]), node=i, k1=False)
            z_op[i] = add("gr", ("ACT", "DVE"), (gm,), node=i, k1=False)
        if i != root:
            rt_op[i] = add("relu", ("DVE", "ACT"), (z_op[i],), node=i)

    pe = add("pe", ("PE",), (z_op[root], id_dma))
    cp = add("cp", ("DVE",), (pe,))
    out = add("dma", ("DMA",), (cp,), what="out", bytes=DC * 4)

    meta = {
        "chunk_of": chunk_of,
        "chunk_sizes": chunk_sizes,
        "n_chunks": n_chunks,
        "n_xch": n_xch,
        "iloc": iloc,
        "z_op": z_op,
    }
    return ops, meta


def _heft(ops):
    """List-schedule the op DAG; sets engine/start on each op, returns
    makespan. Engines: DVE/ACT/POOL/PE compute queues, SP DMA-issue queue,
    DMA transfer pipe."""
    succs = [[] for _ in ops]
    for op in ops:
        for d in op.deps:
            succs[d].append(op.oid)

    def mincost(op):
        if op.kind == "dma":
            return op.info["bytes"] / 360.0 + 600.0
        return min(COST[op.kind].values())

    # upward rank (longest path to sink)
    for op in reversed(ops):
        op.rank = mincost(op) + max(
            (ops[s].rank for s in succs[op.oid]), default=0.0)

    indeg = [len(op.deps) for op in ops]
    import heapq
    ready = [(-op.rank, op.oid) for op in ops if not op.deps]
    heapq.heapify(ready)
    free = {"DVE": 0.0, "ACT": 0.0, "POOL": 0.0, "PE": 0.0,
            "SP": 0.0, "DMA": 0.0}
    load = {k: 0.0 for k in free}
    done_t = [0.0] * len(ops)
    makespan = 0.0
    while ready:
        _, oid = heapq.heappop(ready)
        op = ops[oid]
        if op.kind == "dma":
            ready_t = max((done_t[d] for d in op.deps), default=0.0)
            issue = max(free["SP"], ready_t) + 600.0
            start = max(free["DMA"], issue)
            dur = op.info["bytes"] / 360.0
            free["SP"] = issue
            free["DMA"] = start + dur
            op.engine, op.start, op.dur = "DMA", start, dur
            done_t[oid] = start + dur + DMA_LAT
        else:
            best = None
            for eng in op.engines:
                ready_t = max(
                    (done_t[d] + (0.0 if ops[d].engine == eng else CROSS_LAT)
                     for d in op.deps), default=0.0)
                start = max(free[eng], ready_t)
                dur = COST[op.kind][eng]
                fin = start + dur
                # pressure term steers work off saturated engines even when
                # they are momentarily idle (greedy min-finish alone floods
                # the fastest engine)
                score = fin + 0.55 * (load[eng] + dur)
                if best is None or score < best[0]:
                    best = (score, fin, eng, start)
            _, fin, eng, start = best
            op.engine, op.start, op.dur = eng, start, fin - start
            free[eng] = fin
            load[eng] += op.dur
            done_t[oid] = fin
        makespan = max(makespan, done_t[oid])
        for s in succs[oid]:
            indeg[s] -= 1
            if indeg[s] == 0:
                heapq.heappush(ready, (-ops[s].rank, s))
    return makespan


def _build_program(sched, legalize=True):
    import concourse.bass as bass
    import concourse.tile as tile
    from concourse import mybir

    f32 = mybir.dt.float32
    bf16 = mybir.dt.bfloat16
    MUL = mybir.AluOpType.mult
    MAX = mybir.AluOpType.max
    COPY = mybir.ActivationFunctionType.Copy

    ops, meta = _build_op_graph(sched)
    makespan = _heft(ops)
    order = sorted(range(len(ops)), key=lambda o: (ops[o].start, o))

    internal = sched["internal"]
    leaves = sched["leaves"]
    lab2slot = sched["lab2slot"]
    root = sched["root"]
    iloc = meta["iloc"]
    chunk_of = meta["chunk_of"]
    chunk_sizes = meta["chunk_sizes"]
    n_leaves = max(len(leaves), 1)

    nc = bass.Bass()
    pts = {ci: nc.dram_tensor(f"pc{ci}", [DC, chunk_sizes[ci] * D], bf16,
                              kind="ExternalInput")
           for ci in range(meta["n_chunks"])}
    xr = nc.dram_tensor("xr", [len(internal), D], bf16, kind="ExternalInput")
    rl = nc.dram_tensor("rl", [DC, n_leaves], f32, kind="ExternalInput")
    idm = nc.dram_tensor("idm", [DC, DC], f32, kind="ExternalInput")
    zr = nc.dram_tensor("zr", [1, DC], f32, kind="ExternalOutput")

    with tile.TileContext(nc) as tc:
        with (
            tc.tile_pool(name="pparams", bufs=1) as ppool,
            tc.tile_pool(name="pwork", bufs=1) as wpool,
            tc.tile_pool(name="psmall", bufs=1) as spool,
            tc.tile_pool(name="ppsum", bufs=1, space="PSUM") as psum_pool,
        ):
            pc_t = {}
            xb_t = {}      # per internal-node index t -> AP slice
            rl_t = None
            id_t = None
            rt = {}        # node -> [DC,1] f32 AP of relu'd z
            zt = {}        # node -> [DC,1] f32 tile
            tt_of = {}     # mult op id -> tile
            acc_of = {}    # merge/sttm op id -> tile
            zrow_p = None

            def slot_ap(s):
                ci, off = chunk_of[s]
                return pc_t[ci][:, off * D:(off + 1) * D]

            for oid in order:
                op = ops[oid]
                k = op.kind
                if k == "dma":
                    what = op.info["what"]
                    if what == "param":
                        ci = op.info["chunk"]
                        t = ppool.tile([DC, chunk_sizes[ci] * D], bf16,
                                       tag=f"pc{ci}", name=f"pc{ci}")
                        nc.sync.dma_start(out=t, in_=pts[ci][:, :])
                        pc_t[ci] = t
                    elif what == "xb":
                        rows = op.info["rows"]
                        w = len(rows)
                        xt = ppool.tile([DC, w * D], bf16,
                                        tag=f"xc{op.info['chunk']}",
                                        name=f"xc{op.info['chunk']}")
                        src = xr[rows[0]:rows[0] + w, :]
                        bsrc = bass.AP(
                            tensor=src.tensor, offset=src.offset,
                            ap=[[0, DC]] + list(src.ap),
                        )
                        nc.sync.dma_start(out=xt, in_=bsrc)
                        for j, t in enumerate(rows):
                            xb_t[t] = xt[:, j * D:(j + 1) * D]
                    elif what == "rl":
                        rl_t = spool.tile([DC, n_leaves], f32, tag="rl",
                                          name="rl_t")
                        nc.sync.dma_start(out=rl_t, in_=rl[:, :])
                        for li, leaf in enumerate(leaves):
                            rt[leaf] = rl_t[:, li:li + 1]
                    elif what == "ident":
                        id_t = spool.tile([DC, DC], f32, tag="idm",
                                          name="id_t")
                        nc.sync.dma_start(out=id_t, in_=idm[:, :])
                    elif what == "out":
                        nc.sync.dma_start(out=zr[:, :], in_=zrow_s)
                elif k == "ts":
                    t = wpool.tile([DC, D], bf16, tag="t", name="t", bufs=10)
                    p_ap = slot_ap(op.info["slot"])
                    s_ap = rt[op.info["child"]]
                    if op.engine == "ACT":
                        nc.scalar.mul(t, p_ap, s_ap)
                    elif op.engine == "POOL":
                        nc.gpsimd.tensor_scalar_mul(t, p_ap, s_ap)
                    else:
                        nc.vector.tensor_scalar_mul(t, p_ap, s_ap)
                    tt_of[oid] = t
                elif k == "tt":
                    a = tt_of.get(op.deps[0], acc_of.get(op.deps[0]))
                    b = tt_of.get(op.deps[1], acc_of.get(op.deps[1]))
                    m = wpool.tile([DC, D], bf16, tag="m", name="m", bufs=6)
                    nc.vector.tensor_tensor(out=m, in0=a, in1=b, op=MAX)
                    acc_of[oid] = m
                elif k == "sttm":
                    acc = tt_of.get(op.deps[0], acc_of.get(op.deps[0]))
                    p_ap = slot_ap(op.info["slot"])
                    s_ap = rt[op.info["child"]]
                    af = wpool.tile([DC, D], bf16, tag="af", name="af",
                                    bufs=6)
                    nc.vector.scalar_tensor_tensor(
                        out=af, in0=p_ap, scalar=s_ap, in1=acc,
                        op0=MUL, op1=MAX)
                    acc_of[oid] = af
                elif k in ("sttg", "sttg1"):
                    i = op.info["node"]
                    z = spool.tile([DC, 1], f32, tag=f"z{i}", name=f"z{i}")
                    g = wpool.tile([DC, D], bf16, tag="g", name="g", bufs=4)
                    if k == "sttg":
                        acc = acc_of[op.deps[0]]
                        nc.vector.scalar_tensor_tensor(
                            out=g, in0=acc, scalar=1.0, in1=xb_t[iloc[i]],
                            op0=MUL, op1=MUL, accum_out=z)
                    else:
                        p_ap = slot_ap(op.info["slot"])
                        s_ap = rt[op.info["child"]]
                        nc.vector.scalar_tensor_tensor(
                            out=g, in0=p_ap, scalar=s_ap, in1=xb_t[iloc[i]],
                            op0=MUL, op1=MUL, accum_out=z)
                    zt[i] = z
                elif k == "gm":
                    i = op.info["node"]
                    if op.info["k1"]:
                        a = slot_ap(op.info["slot"])
                    else:
                        a = acc_of.get(op.deps[0], tt_of.get(op.deps[0]))
                    g = wpool.tile([DC, D], bf16, tag="g", name="g", bufs=4)
                    if op.engine == "POOL":
                        nc.gpsimd.tensor_tensor(
                            out=g, in0=a, in1=xb_t[iloc[i]], op=MUL)
                    else:
                        nc.vector.tensor_tensor(
                            out=g, in0=a, in1=xb_t[iloc[i]], op=MUL)
                    tt_of[oid] = g
                elif k == "gr":
                    i = op.info["node"]
                    g = tt_of[op.deps[0]]
                    z = spool.tile([DC, 1], f32, tag=f"z{i}", name=f"z{i}")
                    if op.engine == "DVE":
                        nc.vector.tensor_reduce(
                            z, g, axis=mybir.AxisListType.X,
                            op=mybir.AluOpType.add)
                    else:
                        scr = wpool.tile([DC, D], bf16, tag="scr",
                                         name="scr", bufs=2)
                        if op.info["k1"]:
                            nc.scalar.activation(
                                scr, g, COPY, scale=rt[op.info["child"]],
                                accum_out=z)
                        else:
                            nc.scalar.activation(scr, g, COPY, accum_out=z)
                    zt[i] = z
                elif k == "relu":
                    i = op.info["node"]
                    r = spool.tile([DC, 1], f32, tag=f"r{i}", name=f"r{i}")
                    nc.vector.tensor_scalar_max(r, zt[i], 0.0)
                    rt[i] = r
                elif k == "pe":
                    zrow_p = psum_pool.tile([1, DC], f32, tag="zrow",
                                            name="zrow")
                    nc.tensor.matmul(zrow_p, zt[root], id_t)
                elif k == "cp":
                    zrow_s = spool.tile([1, DC], f32, tag="zrow_s",
                                        name="zrow_s")
                    nc.vector.tensor_copy(zrow_s, zrow_p)

    if legalize:
        _legalize_single_wait(nc)
    return nc, meta, makespan


def _prepare(embeddings, params, children_idx, children_dep, children_mask,
             legalize=True):
    import ml_dtypes

    emb = np.ascontiguousarray(np.asarray(embeddings, dtype=np.float32))
    par = np.asarray(params, dtype=np.float32)
    sched = _schedule(children_idx, children_dep, children_mask)

    key = (
        legalize,
        tuple(sched["order"]),
        tuple(sched["labels"]),
        tuple((i, tuple(e)) for i, e in sched["edges"].items()),
    )
    if key in _CACHE:
        nc, meta = _CACHE[key]
    else:
        nc, meta, _ = _build_program(sched, legalize=legalize)
        _CACHE[key] = (nc, meta)

    internal = sched["internal"]
    leaves = sched["leaves"]
    labels = sched["labels"]
    n_leaves = max(len(leaves), 1)
    chunk_sizes = meta["chunk_sizes"]

    xr = np.ascontiguousarray(emb[internal]).astype(ml_dtypes.bfloat16)
    ident = np.eye(DC, dtype=np.float32)
    in_maps = []
    for kcore in range(N_CORES):
        cols = slice(kcore * DC, (kcore + 1) * DC)
        m = {"xr": xr, "idm": ident}
        off = 0
        for ci, csz in enumerate(chunk_sizes):
            sl = labels[off:off + csz]
            off += csz
            # [csz, D, DC] -> [DC, csz*D] colT layout
            blk = par[sl][:, :, cols].transpose(2, 0, 1).reshape(DC, csz * D)
            m[f"pc{ci}"] = np.ascontiguousarray(blk).astype(ml_dtypes.bfloat16)
        rl_k = np.zeros((DC, n_leaves), dtype=np.float32)
        if leaves:
            rl_k[:, :len(leaves)] = np.maximum(emb[leaves][:, cols], 0.0).T
        m["rl"] = rl_k
        in_maps.append(m)
    return sched, nc, in_maps


def _run(embeddings, params, children_idx, children_dep, children_mask,
         trace=False):
    emb = np.asarray(embeddings, dtype=np.float32)
    cm = np.asarray(children_mask, dtype=bool)
    root = emb.shape[0] - 1
    if not cm[root].any():  # degenerate: root is a leaf
        return emb[root:root + 1].copy(), None

    from concourse.bass_utils import run_bass_kernel_spmd

    sched, nc, in_maps = _prepare(
        embeddings, params, children_idx, children_dep, children_mask
    )
    bkr = run_bass_kernel_spmd(
        nc, in_maps, core_ids=list(range(N_CORES)), trace=trace
    )
    out = np.concatenate(
        [bkr.results[k]["zr"].reshape(DC) for k in range(N_CORES)]
    ).reshape(1, D)
    return out.astype(np.float32), bkr


def kernel(embeddings, params, children_idx, children_dep, children_mask):
    out, _ = _run(embeddings, params, children_idx, children_dep,
                  children_mask)
    return out


def run_traced(embeddings, params, children_idx, children_dep, children_mask):
    return _run(embeddings, params, children_idx, children_dep,
                children_mask, trace=True)
